# revision 18
# baseline (speedup 1.0000x reference)
"""Trainium2 Bass kernel for nn_ContrastLoss.

Reference computation (B=128, P=256 proposals/image, D=1024, K=4 scales):
    box_n = l2norm(box.reshape(B,P,D));  z_n = l2norm(crop)      # [K,B,D]
    cos   = einsum('bpd,kbd->kbp', box_n, z_n)
    mask  = ious >= 0.4  (per (b,p));  cnt_pos = mask.sum(p)
    sim_pos = -(cos*mask).sum(p)/cnt_pos ; sim_neg = -(cos*~mask).sum(p)/cnt_neg
    L[k] = softplus((sim_neg-sim_pos)/T).sum(b);  out = min_k L / B

Key algebraic restructure (per batch b):
    arg[k,b] = (sim_neg-sim_pos)/T = z_n[k,b] . S[b]
    S[b,d]   = sum_p w[b,p] * box[b,p,d]
    w[b,p]   = invnorm[b,p] * coef[b,p]
    coef     = (mask*(1/cnt_pos+1/cnt_neg) - 1/cnt_neg)/T   (iou-only)
so the only heavy pass over the 128 MiB box tensor is one streaming read that
feeds (a) a row-wise sum-of-squares (ScalarE, fused accumulate) and (b) a
PE matmul contraction over proposals with a sparse [128,16] weight matrix.

Work split: coef depends only on ious (128 KiB) and is precomputed on the
host; the device streams box (16 MiB/core, the memory roofline) and returns
S[b,:] per core; the host finishes with the O(K*B*D) z-dot, softplus, batch
sum and min over scales — the same tail it already handled in the baseline.

Sharding: data-parallel over batch. Core c handles batches [16c,16c+16)
(= rows [4096c, 4096c+4096) of box).

Schedule: box chunk 0 is the first DMA issued so the 46.6us stream (the
single-queue DMA roofline in the cost model) starts immediately; the tiny
coef DMA rides behind it. w_sp is zeroed by memset (fp32 tile, bitcast to
fp32r at the matmul) instead of a DMA that would queue behind the stream.
Chunk widths taper (4,...,4,2,1,1 row-tiles) so the post-stream tail is one
row-tile's chain: square -> rsqrt -> weight scatter -> 2 matmuls -> PSUM
copy -> out DMA.
"""

import contextlib
import sys

if "/opt/trn_rl_repo" not in sys.path:
    sys.path.insert(0, "/opt/trn_rl_repo")

import numpy as np

import concourse.bacc as bacc
import concourse.mybir as mybir
import concourse.tile as tile
from concourse.bass_utils import run_bass_kernel_spmd

# Problem constants (hardcoded per harness contract).
B, P, D, K = 128, 256, 1024, 4
N_CORES = 8
B_CORE = B // N_CORES            # 16 batches per core
ROWS = B_CORE * P                # 4096 rows per core
NT = ROWS // 128                 # 32 row-tiles of 128 rows
CHUNK_TILES = (2,) * 15 + (1,)   # row-tiles per DMA chunk; tile 31 arrives
N_SLIV = 4                        # as 4 d-slivers to shorten the tail chain
IOU_THRES = 0.4
TEMP = 0.2
EPS = 1e-12

F32 = mybir.dt.float32
F32R = mybir.dt.float32r
AF = mybir.ActivationFunctionType
ALU = mybir.AluOpType

assert sum(CHUNK_TILES) == NT - 1  # tile 31 streams in as d-slivers


def _emit(tc):
    nc = tc.nc
    box = nc.dram_tensor("box", [ROWS, D], F32, kind="ExternalInput").ap()
    coef_t = nc.dram_tensor("coef_t", [128, NT], F32, kind="ExternalInput").ap()
    s_out_a = nc.dram_tensor("s_out_a", [B_CORE, D], F32, kind="ExternalOutput").ap()
    s_out_b = nc.dram_tensor("s_out_b", [B_CORE, D], F32, kind="ExternalOutput").ap()

    ctx = contextlib.ExitStack()
    with ctx:
        const = ctx.enter_context(tc.tile_pool(name="const", bufs=1))
        boxpool = ctx.enter_context(
            tc.tile_pool(name="boxpool", bufs=len(CHUNK_TILES))
        )
        sqpool = ctx.enter_context(tc.tile_pool(name="sqpool", bufs=2))
        sqppool = ctx.enter_context(tc.tile_pool(name="sqppool", bufs=2))
        psS = ctx.enter_context(tc.tile_pool(name="psS", bufs=2, space="PSUM"))

        # --- box chunk DMAs first: the stream is the critical resource -----
        box3 = box.rearrange("(t p) d -> p t d", p=128)
        chunks = []
        t0 = 0
        for c, w in enumerate(CHUNK_TILES):
            ch = boxpool.tile([128, w * D], F32R, name=f"ch{c}", tag="ch")
            ch3 = ch.rearrange("p (t d) -> p t d", d=D)
            nc.sync.dma_start(ch3, box3[:, t0:t0 + w, :].bitcast(F32R))
            chunks.append((ch, t0, w))
            t0 += w
            if c == 0:
                # tiny (16 KiB) coefficient load rides right behind chunk 0
                coef_sb = const.tile([128, NT], F32)
                nc.sync.dma_start(coef_sb[:], coef_t[:])
        # the last tile arrives as 4 d-slivers so the terminal dependency
        # chain starts from a 256-column square, not a full-tile one
        SL = D // N_SLIV
        slivers = []
        for s in range(N_SLIV):
            sl = boxpool.tile([128, SL], F32R, name=f"sl{s}", tag="sl")
            nc.sync.dma_start(
                sl[:], box3[:, NT - 1, s * SL:(s + 1) * SL].bitcast(F32R)
            )
            slivers.append(sl)

        # sparse per-tile weight columns: w_sp[:, 16*t + t//2] nonzero.
        # fp32r so the BIR verifier sees pre-rounded matmul producers; Memset
        # cannot emit fp32r, so zero a fp32 scratch and convert-copy it in.
        w_sp = const.tile([128, NT * B_CORE], F32R)
        zsc = const.tile([128, NT * B_CORE], F32)
        nc.vector.memset(zsc[:], 0.0)
        nc.vector.tensor_copy(w_sp[:], zsc[:])

        ss_all = const.tile([128, NT], F32)
        rec_all = const.tile([128, NT], F32)
        invn_all = const.tile([128, NT], F32)

        # warm the Sqrt activation table while the first chunk streams in
        warm = const.tile([1, 1], F32)
        nc.vector.memset(warm[:], 1.0)
        nc.scalar.activation(warm[:], warm[:], AF.Sqrt)

        ps_A = psS.tile([B_CORE, D], F32, name="ps_A")
        ps_B = psS.tile([B_CORE, D], F32, name="ps_B")

        # --- main streaming pass over box ---------------------------------
        # Sum-of-squares engine per tile: even tiles on ACT (fused
        # accumulate); odd tiles of the early chunks on Pool(square)+
        # DVE(reduce) so ACT never saturates; t29 on a DVE-only path (the
        # 3.4us Pool chain latency would poison the tail); t30 on ACT.
        # Weights: one batched recip+sqrt per chunk, fused scatter.
        # Matmuls: 256-wide quarters (p-state-reset burst head costs 394ns
        # instead of 788, keeping each burst under the DMA cadence), split
        # into two accumulation groups: A = tiles 0..27 (shipped while the
        # stream still runs), B = tiles 28..31 (short tail group).
        def emit_ss(t, btile):
            if t % 2 == 0:
                sq = sqpool.tile([128, D], F32, name="sq", tag="sq")
                nc.scalar.activation(
                    sq[:], btile, AF.Square, accum_out=ss_all[:, t:t + 1]
                )
            elif t == NT - 3:  # t29: DVE square+reduce
                sqd = sqppool.tile([128, D], F32, name="sqd", tag="sqp")
                nc.vector.tensor_tensor(sqd[:], btile, btile, ALU.mult)
                nc.vector.tensor_reduce(
                    ss_all[:, t:t + 1], sqd[:], mybir.AxisListType.X, ALU.add
                )
            else:
                sqp = sqppool.tile([128, D], F32, name="sqp", tag="sqp")
                nc.gpsimd.tensor_tensor(sqp[:], btile, btile, ALU.mult)
                nc.vector.tensor_reduce(
                    ss_all[:, t:t + 1], sqp[:], mybir.AxisListType.X, ALU.add
                )

        T_A_END = 27  # last tile of output group A

        def emit_mm(t, rhs_of):
            ps = ps_A if t <= T_A_END else ps_B
            lhsT = w_sp[:, t * B_CORE:(t + 1) * B_CORE]
            for h in range(4):
                nc.tensor.matmul(
                    ps[:, h * 256:(h + 1) * 256],
                    lhsT,
                    rhs_of(h),
                    start=(t in (0, T_A_END + 1)),
                    stop=(t in (T_A_END, NT - 1)),
                    skip_group_check=True,
                )

        for c, (ch, t0, w) in enumerate(chunks):
            for rt in range(w):
                emit_ss(t0 + rt, ch[:, rt * D:(rt + 1) * D].bitcast(F32))
            nc.vector.reciprocal(
                rec_all[:, t0:t0 + w], ss_all[:, t0:t0 + w]
            )
            nc.scalar.activation(
                invn_all[:, t0:t0 + w], rec_all[:, t0:t0 + w], AF.Sqrt
            )
            # fused weight+scatter: w_sp[:, 16t + t//2] = coef*invn for the
            # chunk's tiles; an even/odd tile pair is 16 columns apart.
            col = t0 * B_CORE + t0 // 2
            step = B_CORE if w == 2 else 1
            nc.vector.tensor_tensor(
                w_sp[:, col:col + step * (w - 1) + 1:step],
                invn_all[:, t0:t0 + w],
                coef_sb[:, t0:t0 + w],
                ALU.mult,
            )
            for rt in range(w):
                t = t0 + rt
                emit_mm(t, lambda h, rt=rt: ch[
                    :, rt * D + h * 256:rt * D + (h + 1) * 256
                ])

        # group A is complete mid-stream: copy + ship while B still runs
        sa_sb = const.tile([B_CORE, D], F32)
        nc.vector.tensor_copy(sa_sb[:, 0:512], ps_A[:, 0:512])
        nc.vector.tensor_copy(sa_sb[:, 512:1024], ps_A[:, 512:1024])
        nc.sync.dma_start(s_out_a[:], sa_sb[:])

        # --- tile 31 from slivers: short terminal chain --------------------
        tL = NT - 1
        ss_sl = const.tile([128, N_SLIV], F32)
        for s in range(N_SLIV):
            bt = slivers[s].bitcast(F32)
            if s < 2:
                sq = sqpool.tile([128, SL], F32, name="sqs", tag="sq")
                nc.scalar.activation(
                    sq[:], bt, AF.Square, accum_out=ss_sl[:, s:s + 1]
                )
            else:
                sqp = sqppool.tile([128, SL], F32, name="sqsp", tag="sqp")
                nc.gpsimd.tensor_tensor(sqp[:], bt, bt, ALU.mult)
                nc.vector.tensor_reduce(
                    ss_sl[:, s:s + 1], sqp[:], mybir.AxisListType.X, ALU.add
                )
        nc.vector.tensor_reduce(
            ss_all[:, tL:tL + 1], ss_sl[:], mybir.AxisListType.X, ALU.add
        )
        nc.vector.reciprocal(rec_all[:, tL:tL + 1], ss_all[:, tL:tL + 1])
        nc.scalar.activation(
            invn_all[:, tL:tL + 1], rec_all[:, tL:tL + 1], AF.Sqrt
        )
        colL = tL * B_CORE + tL // 2
        nc.vector.tensor_tensor(
            w_sp[:, colL:colL + 1], invn_all[:, tL:tL + 1],
            coef_sb[:, tL:tL + 1], ALU.mult,
        )
        emit_mm(tL, lambda h: slivers[h][:])

        # --- tail: group B PSUM -> SBUF halves on DVE/ACT, out DMA ---------
        sb_sb = const.tile([B_CORE, D], F32)
        nc.vector.tensor_copy(sb_sb[:, 0:512], ps_B[:, 0:512])
        nc.scalar.activation(sb_sb[:, 512:1024], ps_B[:, 512:1024], AF.Copy)
        nc.sync.dma_start(s_out_b[:], sb_sb[:])


_NC_CACHE = None


def _get_nc():
    global _NC_CACHE
    if _NC_CACHE is None:
        nc = bacc.Bacc(
            "TRN2", target_bir_lowering=False, debug=False, num_devices=N_CORES
        )
        with tile.TileContext(nc) as tc:
            _emit(tc)
        nc.compile()
        _NC_CACHE = nc
    return _NC_CACHE


def _coef_full(ious):
    """Per-row matmul coefficient (mask*(1/cp+1/cn) - 1/cn)/T, [B, P] f32."""
    iou = np.asarray(ious, dtype=np.float32).reshape(B, P)
    mask = iou >= IOU_THRES
    cp = mask.sum(axis=1).astype(np.float32)
    cn = np.float32(P) - cp
    rp = np.float32(1.0) / cp
    rn = np.float32(1.0) / cn
    coef = (mask * (rp + rn)[:, None] - rn[:, None]) / np.float32(TEMP)
    return coef.astype(np.float32).reshape(B * P)


def _in_maps(box_cls_feat_con, ious):
    box = np.ascontiguousarray(np.asarray(box_cls_feat_con, dtype=np.float32))
    coef = _coef_full(ious)
    maps = []
    for c in range(N_CORES):
        rows = slice(c * ROWS, (c + 1) * ROWS)
        maps.append({
            "box": np.ascontiguousarray(box[rows]),
            "coef_t": np.ascontiguousarray(coef[rows].reshape(NT, 128).T),
        })
    return maps


def kernel(box_cls_feat_con, crop_feat_con, batch_size, ious, _trace=False):
    nc = _get_nc()
    maps = _in_maps(box_cls_feat_con, ious)
    res = run_bass_kernel_spmd(nc, maps, core_ids=list(range(N_CORES)), trace=_trace)

    # host finishing: z normalization, per-batch dots, softplus, min over k
    crop = np.asarray(crop_feat_con, dtype=np.float64)  # [K, B, D]
    z_n = crop / np.maximum(np.linalg.norm(crop, axis=-1, keepdims=True), EPS)
    l_total = np.zeros(K, dtype=np.float64)
    for c in range(N_CORES):
        S = (
            res.results[c]["s_out_a"].astype(np.float64)
            + res.results[c]["s_out_b"].astype(np.float64)
        )  # [B_CORE, D]
        z = z_n[:, c * B_CORE:(c + 1) * B_CORE, :]      # [K, B_CORE, D]
        args = np.einsum("kbd,bd->kb", z, S)
        l_total += np.log1p(np.exp(args)).sum(axis=1)
    out = np.float32(l_total.min() / float(B))
    if _trace:
        kernel._last_results = res
    return np.asarray(out, dtype=np.float32)


# revision 24
# speedup vs baseline: 1.0615x; 1.0615x over previous
"""Trainium2 Bass kernel for nn_ContrastLoss.

Reference computation (B=128, P=256 proposals/image, D=1024, K=4 scales):
    box_n = l2norm(box.reshape(B,P,D));  z_n = l2norm(crop)      # [K,B,D]
    cos   = einsum('bpd,kbd->kbp', box_n, z_n)
    mask  = ious >= 0.4  (per (b,p));  cnt_pos = mask.sum(p)
    sim_pos = -(cos*mask).sum(p)/cnt_pos ; sim_neg = -(cos*~mask).sum(p)/cnt_neg
    L[k] = softplus((sim_neg-sim_pos)/T).sum(b);  out = min_k L / B

Key algebraic restructure (per batch b):
    arg[k,b] = (sim_neg-sim_pos)/T = z_n[k,b] . S[b]
    S[b,d]   = sum_p w[b,p] * box[b,p,d]
    w[b,p]   = invnorm[b,p] * coef[b,p]
    coef     = (mask*(1/cnt_pos+1/cnt_neg) - 1/cnt_neg)/T   (iou-only)
so the only heavy pass over the 128 MiB box tensor is one streaming read that
feeds (a) a row-wise sum-of-squares (ScalarE, fused accumulate) and (b) a
PE matmul contraction over proposals with a sparse [128,16] weight matrix.

Work split: coef depends only on ious (128 KiB) and is precomputed on the
host; the device streams box (16 MiB/core, the memory roofline) and returns
S[b,:] per core; the host finishes with the O(K*B*D) z-dot, softplus, batch
sum and min over scales — the same tail it already handled in the baseline.

Sharding: data-parallel over batch. Core c handles batches [16c,16c+16)
(= rows [4096c, 4096c+4096) of box).

Schedule: box chunk 0 is the first DMA issued so the 46.6us stream (the
single-queue DMA roofline in the cost model) starts immediately; the tiny
coef DMA rides behind it. w_sp is zeroed by memset (fp32 tile, bitcast to
fp32r at the matmul) instead of a DMA that would queue behind the stream.
Chunk widths taper (4,...,4,2,1,1 row-tiles) so the post-stream tail is one
row-tile's chain: square -> rsqrt -> weight scatter -> 2 matmuls -> PSUM
copy -> out DMA.
"""

import contextlib
import sys

if "/opt/trn_rl_repo" not in sys.path:
    sys.path.insert(0, "/opt/trn_rl_repo")

import numpy as np

import concourse.bacc as bacc
import concourse.mybir as mybir
import concourse.tile as tile
from concourse.bass_utils import run_bass_kernel_spmd

# Problem constants (hardcoded per harness contract).
B, P, D, K = 128, 256, 1024, 4
N_CORES = 8
B_CORE = B // N_CORES            # 16 batches per core
ROWS = B_CORE * P                # 4096 rows per core
NT = ROWS // 128                 # 32 row-tiles of 128 rows
CHUNK_TILES = (2,) * 15 + (1,)   # row-tiles per DMA chunk; tile 31 arrives
N_SLIV = 4                        # as 4 d-slivers to shorten the tail chain
IOU_THRES = 0.4
TEMP = 0.2
EPS = 1e-12

F32 = mybir.dt.float32
F32R = mybir.dt.float32r
AF = mybir.ActivationFunctionType
ALU = mybir.AluOpType

assert sum(CHUNK_TILES) == NT - 1  # tile 31 streams in as d-slivers


def _emit(tc):
    nc = tc.nc
    box = nc.dram_tensor("box", [ROWS, D], F32, kind="ExternalInput").ap()
    coef_t = nc.dram_tensor("coef_t", [128, NT], F32, kind="ExternalInput").ap()
    s_out = nc.dram_tensor("s_out", [B_CORE, D], F32, kind="ExternalOutput").ap()

    ctx = contextlib.ExitStack()
    with ctx:
        const = ctx.enter_context(tc.tile_pool(name="const", bufs=1))
        boxpool = ctx.enter_context(
            tc.tile_pool(name="boxpool", bufs=len(CHUNK_TILES))
        )
        sqpool = ctx.enter_context(tc.tile_pool(name="sqpool", bufs=2))
        sqppool = ctx.enter_context(tc.tile_pool(name="sqppool", bufs=2))
        psS = ctx.enter_context(tc.tile_pool(name="psS", bufs=2, space="PSUM"))

        # --- box chunk DMAs first: the stream is the critical resource -----
        box3 = box.rearrange("(t p) d -> p t d", p=128)
        chunks = []
        t0 = 0
        for c, w in enumerate(CHUNK_TILES):
            ch = boxpool.tile([128, w * D], F32R, name=f"ch{c}", tag="ch")
            ch3 = ch.rearrange("p (t d) -> p t d", d=D)
            nc.sync.dma_start(ch3, box3[:, t0:t0 + w, :].bitcast(F32R))
            chunks.append((ch, t0, w))
            t0 += w
            if c == 0:
                # tiny (16 KiB) coefficient load rides right behind chunk 0
                coef_sb = const.tile([128, NT], F32)
                nc.sync.dma_start(coef_sb[:], coef_t[:])
        # the last tile arrives as 4 d-slivers so the terminal dependency
        # chain starts from a 256-column square, not a full-tile one
        SL = D // N_SLIV
        slivers = []
        for s in range(N_SLIV):
            sl = boxpool.tile([128, SL], F32R, name=f"sl{s}", tag="sl")
            nc.sync.dma_start(
                sl[:], box3[:, NT - 1, s * SL:(s + 1) * SL].bitcast(F32R)
            )
            slivers.append(sl)

        # sparse per-tile weight columns: w_sp[:, 16*t + t//2] nonzero.
        # fp32r so the BIR verifier sees pre-rounded matmul producers; Memset
        # cannot emit fp32r, so zero a fp32 scratch and convert-copy it in.
        w_sp = const.tile([128, NT * B_CORE], F32R)
        zsc = const.tile([128, NT * B_CORE], F32)
        nc.vector.memset(zsc[:], 0.0)
        nc.vector.tensor_copy(w_sp[:], zsc[:])

        ss_all = const.tile([128, NT], F32)
        rec_all = const.tile([128, NT], F32)
        invn_all = const.tile([128, NT], F32)

        # warm the Sqrt activation table while the first chunk streams in
        warm = const.tile([1, 1], F32)
        nc.vector.memset(warm[:], 1.0)
        nc.scalar.activation(warm[:], warm[:], AF.Sqrt)

        ps_S = psS.tile([B_CORE, D], F32)

        # --- main streaming pass over box ---------------------------------
        # Sum-of-squares engine per tile: even tiles on ACT (fused
        # accumulate); odd tiles of the early chunks on Pool(square)+
        # DVE(reduce) so ACT never saturates; t29 on a DVE-only path (the
        # 3.4us Pool chain latency would poison the tail); t30 on ACT.
        # Weights: one batched recip+sqrt per chunk, fused scatter.
        # Matmuls: 256-wide quarters (p-state-reset burst head costs 394ns
        # instead of 788, keeping each burst under the DMA cadence), split
        # into two accumulation groups: A = tiles 0..27 (shipped while the
        # stream still runs), B = tiles 28..31 (short tail group).
        def emit_ss(t, btile):
            if t % 2 == 0:
                sq = sqpool.tile([128, D], F32, name="sq", tag="sq")
                nc.scalar.activation(
                    sq[:], btile, AF.Square, accum_out=ss_all[:, t:t + 1]
                )
            elif t == NT - 3:  # t29: DVE square+reduce
                sqd = sqppool.tile([128, D], F32, name="sqd", tag="sqp")
                nc.vector.tensor_tensor(sqd[:], btile, btile, ALU.mult)
                nc.vector.tensor_reduce(
                    ss_all[:, t:t + 1], sqd[:], mybir.AxisListType.X, ALU.add
                )
            else:
                sqp = sqppool.tile([128, D], F32, name="sqp", tag="sqp")
                nc.gpsimd.tensor_tensor(sqp[:], btile, btile, ALU.mult)
                nc.vector.tensor_reduce(
                    ss_all[:, t:t + 1], sqp[:], mybir.AxisListType.X, ALU.add
                )

        def emit_mm(t, rhs_of):
            lhsT = w_sp[:, t * B_CORE:(t + 1) * B_CORE]
            for h in range(4):
                nc.tensor.matmul(
                    ps_S[:, h * 256:(h + 1) * 256],
                    lhsT,
                    rhs_of(h),
                    start=(t == 0),
                    stop=(t == NT - 1),
                    skip_group_check=True,
                )

        for c, (ch, t0, w) in enumerate(chunks):
            for rt in range(w):
                emit_ss(t0 + rt, ch[:, rt * D:(rt + 1) * D].bitcast(F32))
            nc.vector.reciprocal(
                rec_all[:, t0:t0 + w], ss_all[:, t0:t0 + w]
            )
            nc.scalar.activation(
                invn_all[:, t0:t0 + w], rec_all[:, t0:t0 + w], AF.Sqrt
            )
            # fused weight+scatter: w_sp[:, 16t + t//2] = coef*invn for the
            # chunk's tiles; an even/odd tile pair is 16 columns apart.
            col = t0 * B_CORE + t0 // 2
            step = B_CORE if w == 2 else 1
            nc.vector.tensor_tensor(
                w_sp[:, col:col + step * (w - 1) + 1:step],
                invn_all[:, t0:t0 + w],
                coef_sb[:, t0:t0 + w],
                ALU.mult,
            )
            for rt in range(w):
                t = t0 + rt
                emit_mm(t, lambda h, rt=rt: ch[
                    :, rt * D + h * 256:rt * D + (h + 1) * 256
                ])

        # --- tile 31 from slivers: short terminal chain --------------------
        tL = NT - 1
        ss_sl = const.tile([128, N_SLIV], F32)
        for s in range(N_SLIV):
            bt = slivers[s].bitcast(F32)
            if s < 2:
                sq = sqpool.tile([128, SL], F32, name="sqs", tag="sq")
                nc.scalar.activation(
                    sq[:], bt, AF.Square, accum_out=ss_sl[:, s:s + 1]
                )
            else:
                sqp = sqppool.tile([128, SL], F32, name="sqsp", tag="sqp")
                nc.gpsimd.tensor_tensor(sqp[:], bt, bt, ALU.mult)
                nc.vector.tensor_reduce(
                    ss_sl[:, s:s + 1], sqp[:], mybir.AxisListType.X, ALU.add
                )
        nc.vector.tensor_reduce(
            ss_all[:, tL:tL + 1], ss_sl[:], mybir.AxisListType.X, ALU.add
        )
        nc.vector.reciprocal(rec_all[:, tL:tL + 1], ss_all[:, tL:tL + 1])
        nc.scalar.activation(
            invn_all[:, tL:tL + 1], rec_all[:, tL:tL + 1], AF.Sqrt
        )
        colL = tL * B_CORE + tL // 2
        nc.vector.tensor_tensor(
            w_sp[:, colL:colL + 1], invn_all[:, tL:tL + 1],
            coef_sb[:, tL:tL + 1], ALU.mult,
        )
        emit_mm(tL, lambda h: slivers[h][:])

        # --- tail: PSUM -> SBUF halves on DVE/ACT, one out DMA -------------
        s_sb = const.tile([B_CORE, D], F32)
        nc.vector.tensor_copy(s_sb[:, 0:512], ps_S[:, 0:512])
        nc.scalar.activation(s_sb[:, 512:1024], ps_S[:, 512:1024], AF.Copy)
        nc.sync.dma_start(s_out[:], s_sb[:])


_NC_CACHE = None


def _get_nc():
    global _NC_CACHE
    if _NC_CACHE is None:
        nc = bacc.Bacc(
            "TRN2", target_bir_lowering=False, debug=False, num_devices=N_CORES
        )
        with tile.TileContext(nc) as tc:
            _emit(tc)
        nc.compile()
        _NC_CACHE = nc
    return _NC_CACHE


def _coef_full(ious):
    """Per-row matmul coefficient (mask*(1/cp+1/cn) - 1/cn)/T, [B, P] f32."""
    iou = np.asarray(ious, dtype=np.float32).reshape(B, P)
    mask = iou >= IOU_THRES
    cp = mask.sum(axis=1).astype(np.float32)
    cn = np.float32(P) - cp
    rp = np.float32(1.0) / cp
    rn = np.float32(1.0) / cn
    coef = (mask * (rp + rn)[:, None] - rn[:, None]) / np.float32(TEMP)
    return coef.astype(np.float32).reshape(B * P)


def _in_maps(box_cls_feat_con, ious):
    box = np.ascontiguousarray(np.asarray(box_cls_feat_con, dtype=np.float32))
    coef = _coef_full(ious)
    maps = []
    for c in range(N_CORES):
        rows = slice(c * ROWS, (c + 1) * ROWS)
        maps.append({
            "box": np.ascontiguousarray(box[rows]),
            "coef_t": np.ascontiguousarray(coef[rows].reshape(NT, 128).T),
        })
    return maps


def kernel(box_cls_feat_con, crop_feat_con, batch_size, ious, _trace=False):
    nc = _get_nc()
    maps = _in_maps(box_cls_feat_con, ious)
    res = run_bass_kernel_spmd(nc, maps, core_ids=list(range(N_CORES)), trace=_trace)

    # host finishing: z normalization, per-batch dots, softplus, min over k
    crop = np.asarray(crop_feat_con, dtype=np.float64)  # [K, B, D]
    z_n = crop / np.maximum(np.linalg.norm(crop, axis=-1, keepdims=True), EPS)
    l_total = np.zeros(K, dtype=np.float64)
    for c in range(N_CORES):
        S = res.results[c]["s_out"].astype(np.float64)  # [B_CORE, D]
        z = z_n[:, c * B_CORE:(c + 1) * B_CORE, :]      # [K, B_CORE, D]
        args = np.einsum("kbd,bd->kb", z, S)
        l_total += np.log1p(np.exp(args)).sum(axis=1)
    out = np.float32(l_total.min() / float(B))
    if _trace:
        kernel._last_results = res
    return np.asarray(out, dtype=np.float32)
